# revision 28
# baseline (speedup 1.0000x reference)
"""Bidirectional LSTM (B=64, T=256, D=512, U=500) on 8 Trainium2 NeuronCores.

Sharding: 2 directions x 4 batch-groups -> 16 samples per core, one direction
per core. Backward cores receive time-reversed x from the host; the device
program is pure SPMD.

v4 design: recurrence in (128,32) column-tiled PE mode, elementwise tail in
transposed space, and the input GEMM interleaved into the recurrence loop:
  - z bank [128,500]: f@0-15, i@32-47, o@64-79, g@96-111 (bf16 operands),
    built by 4-way concurrent col-tiled matmuls (xz via selector-identity MM
    with start=True, then 4 recurrent u-chunks accumulate)
  - tanh fold: host scales g columns of Wk/Wr/b by 2; ONE packed sigmoid over
    z[0:112] covers all four gates; a constant-1.0 row block (s rows 112-127)
    plus a -1 selector column turns 2*sigmoid into tanh during the transpose
  - transposes: per u-chunk, 4 concurrent piece-matmuls with a [128,64]
    block-selector rhs -> T[128,4,64] = fT|iT|oT|gT
  - T-space tail: gT copied to SBUF on ScalarE; t1 = iT*gT; t2 = fT*cT;
    cT = t1+t2; th = tanh(cT) and hT = oT*th in chunk-halves
  - input GEMM xz = x@Wk + b: 3 m-tiles as a serial prefix, then one
    [128rows x 500cols] bank-tile every other step fills the PE's
    dependency-wait gaps and keeps the HAM clock gate warm; bias-adds run on
    the otherwise-idle GpSimd; xz round-trips through DRAM
"""

import numpy as np

B, T, D, U = 64, 256, 512, 500
G4 = 4 * U            # 2000
NCORES = 8
BC = B // 4           # 16 samples per core
UP = 512              # padded U (4 chunks of 128)
NCH = 4               # u-chunks
CH_W = [128, 128, 128, 116]
CH_LO = [0, 128, 256, 384]
NSL = 500
DCH = 4               # D = 4 chunks of 128 (input GEMM contraction)
MT = (T * BC) // 128  # 32 M-tiles in the input GEMM
NQ = 4 * MT           # 128 GEMM bank-tiles (m-tile x n-bank)
PREFIX_Q = 12         # 3 m-tiles computed before the loop

_CACHE = {}


def _build_program(steps=T):
    import concourse.bass as bass
    import concourse.bacc as bacc
    import concourse.tile as tile
    import concourse.mybir as mybir

    dt = mybir.dt
    AF = mybir.ActivationFunctionType
    f32 = dt.float32
    f32r = dt.float32r
    bf16 = dt.bfloat16

    nc = bacc.Bacc("TRN2")

    xT = nc.dram_tensor("xT", [D, T * BC], bf16, kind="ExternalInput")
    h0T = nc.dram_tensor("h0T", [128, NCH, BC], bf16, kind="ExternalInput")
    c0T = nc.dram_tensor("c0T", [128, NCH, BC], f32, kind="ExternalInput")
    Wk = nc.dram_tensor("Wk", [D, G4], bf16, kind="ExternalInput")  # cols fiog
    Wr = nc.dram_tensor("Wr", [UP, G4], bf16, kind="ExternalInput")  # rows pad
    bv = nc.dram_tensor("b", [G4], f32, kind="ExternalInput")
    selc = nc.dram_tensor("selc", [128, BC], bf16, kind="ExternalInput")
    sela = nc.dram_tensor("sela", [128, 4 * BC], bf16, kind="ExternalInput")
    # yT[t, p, r, b] = h_t[b, 128*r + p]  (junk rows p>=116 for r=3)
    yT = nc.dram_tensor("yT", [T, 128, NCH, BC], bf16, kind="ExternalOutput")
    xz = nc.dram_tensor("xzbuf", [T * BC, G4], bf16)

    with tile.TileContext(nc) as tc:
        with tc.tile_pool(name="persist", bufs=1) as persist, \
             tc.tile_pool(name="gout", bufs=3) as gout, \
             tc.tile_pool(name="state", bufs=2) as st, \
             tc.tile_pool(name="gates", bufs=2) as gt, \
             tc.tile_pool(name="xzin", bufs=4) as xzp, \
             tc.tile_pool(name="gps", bufs=2, space="PSUM") as gps, \
             tc.tile_pool(name="zps", bufs=2, space="PSUM") as zp, \
             tc.tile_pool(name="tps", bufs=2, space="PSUM") as tpp:

            wr_sb = persist.tile([128, NCH, G4], bf16)
            sel_c = persist.tile([128, BC], bf16)
            nc.sync.dma_start(sel_c, selc[:, :])
            sel_a = persist.tile([128, 4 * BC], bf16)
            nc.sync.dma_start(sel_a, sela[:, :])
            wk_sb = persist.tile([128, DCH, G4], bf16)
            for k in range(DCH):
                nc.gpsimd.dma_start(wk_sb[:, k, :], Wk[k * 128:(k + 1) * 128, :])
            XB = (T * BC) // 8
            xtb = []
            for hblk in range(8):
                xtb_t = persist.tile([128, DCH, XB], bf16, name=f"xtb{hblk}")
                xtb.append(xtb_t)
                for k in range(DCH):
                    nc.gpsimd.dma_start(
                        xtb_t[:, k, :],
                        xT[k * 128:(k + 1) * 128, hblk * XB:(hblk + 1) * XB])
            for r in range(NCH):
                nc.gpsimd.dma_start(wr_sb[:, r, :], Wr[r * 128:(r + 1) * 128, :])


            def gemm_bank_tile(q):
                """One [128 x 500] tile of xz = x@Wk (m-tile q//4, bank q%4).

                Column-tiled bf16 (stays in (128,32) mode). The k-order is
                staggered across col groups because start=True clears the
                whole bank's has_written bits: only the very first matmul may
                use it; later groups' k0 land on cleared bits and overwrite.
                Bias is added later by the xz selector matmul (b row in xzt).
                Evacuation is a ScalarE copy issued at the end of the step.
                """
                m, n = q // 4, q % 4
                gb = gps.tile([128, 512], f32, tag="gb", name=f"gb{q}")
                mlo = (m % 4) * 128
                # k staggered across col groups: concurrent tiles then stream
                # DIFFERENT wk addresses (same-address streams halve the rate
                # on the SBUF read ports). start=True clears has_written for
                # [out partitions x whole bank free-range]; groups have
                # disjoint partitions so each primes at its own k=0.
                for s in range(DCH + 3):
                    for j in range(4):
                        k = s - j
                        if k < 0 or k >= DCH:
                            continue
                        nc.tensor.matmul(
                            gb[32 * j:32 * (j + 1), 0:NSL],
                            lhsT=xtb[m // 4][:, k, mlo + 32 * j:mlo + 32 * (j + 1)],
                            rhs=wk_sb[:, k, n * NSL:(n + 1) * NSL],
                            start=(k == 0),
                            stop=(k == DCH - 1),
                            tile_position=(0, 32 * j),
                        )
                so = gout.tile([128, NSL], bf16, tag="so", name=f"so{q}")
                for e in range(4):
                    el, eh = 125 * e, 125 * (e + 1)
                    nc.vector.tensor_copy(so[:, el:eh], gb[:, el:eh])
                nc.sync.dma_start(
                    xz[m * 128:(m + 1) * 128, n * NSL:(n + 1) * NSL], so)

            # ---- init: memset junk regions of rotating buffers once ----
            for _ in range(2):
                zi = zp.tile([128, 384], f32, tag="zA")
                nc.vector.memset(zi, 0.0)
                zi2 = zp.tile([128, NSL - 384], f32, tag="zB")
                nc.vector.memset(zi2, 0.0)
            for _ in range(2):
                ti = tpp.tile([128, NCH, 4 * BC], f32, tag="T")
                nc.vector.memset(ti, 0.0)
            for _ in range(2):
                si = gt.tile([128, NSL], bf16, tag="s")
                nc.vector.memset(si, 0.0)
                nc.vector.memset(si[96:128, :], 1.0)
            for _ in range(4):
                xi = xzp.tile([128, G4], bf16, tag="xz")
                nc.vector.memset(xi, 0.0)
                # bias lives in row 16 of every xz staging buffer; the
                # selector's ones-row adds it to all batch rows
                nc.gpsimd.dma_start(xi[16:17, :], bv[:].unsqueeze(0))

            cT_prev = st.tile([128, NCH, BC], f32, tag="cT")
            nc.sync.dma_start(cT_prev, c0T[:, :, :])
            hT_prev = st.tile([128, NCH, BC], bf16, tag="hT")
            nc.sync.dma_start(hT_prev, h0T[:, :, :])

            # ---- GEMM prefix ----
            for q in range(PREFIX_Q):
                gemm_bank_tile(q)
            next_q = PREFIX_Q

            xzt = xzp.tile([128, G4], bf16, tag="xz")
            nc.sync.dma_start(xzt[0:BC, :], xz[0:BC, :])

            # z built in column halves A=[0:256], B=[256:500]: sigmoid's first
            # slice starts right after the A half-burst
            HALVES = [(0, 384), (384, NSL)]

            def xz_slot(zh, xzt_, c0, c1):
                for g in range(4):
                    nc.tensor.matmul(
                        zh[32 * g:32 * g + BC, 0:c1 - c0],
                        lhsT=sel_c,
                        rhs=xzt_[:, g * NSL + c0:g * NSL + c1],
                        start=True, stop=False,
                        tile_position=(0, 32 * g),
                    )

            def rec_slot(zh, hT_, r, c0, c1):
                for g in range(4):
                    nc.tensor.matmul(
                        zh[32 * g:32 * g + BC, 0:c1 - c0],
                        lhsT=hT_[:, r, :],
                        rhs=wr_sb[:, r, g * NSL + c0:g * NSL + c1],
                        start=False, stop=(r == NCH - 1),
                        tile_position=(0, 32 * g),
                    )

            def new_z():
                return (zp.tile([128, 384], f32, tag="zA", name="zA"),
                        zp.tile([128, NSL - 384], f32, tag="zB", name="zB"))

            z = new_z()
            for (c0, c1), zh in zip(HALVES, z):
                xz_slot(zh, xzt, c0, c1)
                for r in range(NCH):
                    rec_slot(zh, hT_prev, r, c0, c1)

            for t in range(steps):
                if t + 1 < steps:
                    xzt_n = xzp.tile([128, G4], bf16, tag="xz")
                    nc.sync.dma_start(
                        xzt_n[0:BC, :], xz[(t + 1) * BC:(t + 2) * BC, :])

                # packed sigmoid over all 4 gates, one op per z half-tile
                s_sb = gt.tile([128, NSL], bf16, tag="s")
                nc.scalar.activation(
                    s_sb[0:112, 0:384], z[0][0:112, :], AF.Sigmoid)
                nc.scalar.activation(
                    s_sb[0:112, 384:NSL], z[1][0:112, :], AF.Sigmoid)

                if t + 1 < steps:
                    z_n = new_z()
                    for (c0, c1), zh in zip(HALVES, z_n):
                        xz_slot(zh, xzt_n, c0, c1)

                # transposes: per u-chunk, 4 concurrent piece-MMs
                Tt = tpp.tile([128, NCH, 4 * BC], f32, tag="T")
                for r in range(NCH):
                    lo, w = CH_LO[r], CH_W[r]
                    for j in range(4):
                        pw = min(32, w - 32 * j)
                        if pw <= 0:
                            continue
                        pl = lo + 32 * j
                        nc.tensor.matmul(
                            Tt[32 * j:32 * j + pw, r, :],
                            lhsT=s_sb[:, pl:pl + pw],
                            rhs=sel_a,
                            start=True, stop=True,
                            tile_position=(0, 32 * j),
                        )

                # T-space tail, sliced into chunk-halves so the first half's
                # hT unblocks next step's rec chunks 0,1 early
                fT = Tt[:, :, 0:BC]
                iT = Tt[:, :, BC:2 * BC]
                oT = Tt[:, :, 2 * BC:3 * BC]
                gT = Tt[:, :, 3 * BC:4 * BC]
                g_sb = gt.tile([128, NCH, BC], f32, tag="gsb")
                nc.scalar.activation(g_sb, gT, AF.Copy)
                t2 = gt.tile([128, NCH, BC], f32, tag="t2")
                nc.vector.tensor_mul(t2, fT, cT_prev)
                t1 = gt.tile([128, NCH, BC], f32, tag="t1")
                nc.vector.tensor_mul(t1, iT, g_sb)
                cT_new = st.tile([128, NCH, BC], f32, tag="cT")
                nc.vector.tensor_add(cT_new[:, 0:2, :], t1[:, 0:2, :],
                                     t2[:, 0:2, :])
                nc.vector.tensor_add(cT_new[:, 2:4, :], t1[:, 2:4, :],
                                     t2[:, 2:4, :])
                th = gt.tile([128, NCH, BC], f32, tag="th")
                hT_new = st.tile([128, NCH, BC], bf16, tag="hT")
                nc.scalar.activation(th[:, 0:2, :], cT_new[:, 0:2, :], AF.Tanh)
                nc.vector.tensor_mul(hT_new[:, 0:2, :], oT[:, 0:2, :],
                                     th[:, 0:2, :])
                nc.scalar.activation(th[:, 2:4, :], cT_new[:, 2:4, :], AF.Tanh)
                nc.vector.tensor_mul(hT_new[:, 2:4, :], oT[:, 2:4, :],
                                     th[:, 2:4, :])

                nc.sync.dma_start(yT[t], hT_new)

                if t + 1 < steps:
                    for (c0, c1), zh in zip(HALVES, z_n):
                        for r in range(NCH):
                            rec_slot(zh, hT_new, r, c0, c1)
                    z = z_n
                    xzt = xzt_n

                # one GEMM bank-tile every other step; issued last so its PE
                # matmuls and DVE bias-add drain into idle time
                if t % 2 == 0 and next_q < NQ:
                    gemm_bank_tile(next_q)
                    next_q += 1

                cT_prev = cT_new
                hT_prev = hT_new
    nc.finalize()
    return nc


# Keras gate order [i, f, g, o] -> kernel order [f, i, o, g]
_PERM = np.concatenate([
    np.arange(U, 2 * U),      # f
    np.arange(0, U),          # i
    np.arange(3 * U, 4 * U),  # o
    np.arange(2 * U, 3 * U),  # g
])
# scale applied per gate column block after permutation: g block doubled
_GSCALE = np.concatenate([
    np.ones(3 * U, np.float32), np.full(U, 2.0, np.float32)])


def _make_in_maps(x, h_f, c_f, h_b, c_b, Wk_f, Wr_f, b_f, Wk_b, Wr_b, b_b):
    import ml_dtypes
    bf16 = ml_dtypes.bfloat16

    x = np.ascontiguousarray(np.asarray(x, np.float32))
    Wks = [np.ascontiguousarray(
               (np.asarray(Wk_f, np.float32)[:, _PERM] * _GSCALE).astype(bf16)),
           np.ascontiguousarray(
               (np.asarray(Wk_b, np.float32)[:, _PERM] * _GSCALE).astype(bf16))]
    Wrs = []
    for Wr in (Wr_f, Wr_b):
        w = np.asarray(Wr, np.float32)[:, _PERM] * _GSCALE
        wp = np.zeros((UP, G4), np.float32)
        wp[:U] = w
        Wrs.append(np.ascontiguousarray(wp.astype(bf16)))
    bs = [np.ascontiguousarray(np.asarray(b_f, np.float32)[_PERM] * _GSCALE),
          np.ascontiguousarray(np.asarray(b_b, np.float32)[_PERM] * _GSCALE)]

    selc = np.zeros((128, BC), np.float32)
    for n in range(BC):
        selc[n, n] = 1.0
    selc[16, :] = 1.0  # adds the b row (xzt row 16) to every batch row
    sela = np.zeros((128, 4 * BC), np.float32)
    for n in range(BC):
        sela[n, n] = 1.0            # fT
        sela[32 + n, BC + n] = 1.0  # iT
        sela[64 + n, 2 * BC + n] = 1.0  # oT
        sela[96 + n, 3 * BC + n] = 2.0  # 2*sg ...
        sela[112, 3 * BC + n] = -1.0  # ... - 1 (s_sb rows 112+ are const 1.0)
    selc = selc.astype(bf16)
    sela = sela.astype(bf16)

    def to_T(hv, dtype):
        hp = np.zeros((BC, UP), np.float32)
        hp[:, :U] = hv
        return np.ascontiguousarray(
            hp.reshape(BC, NCH, 128).transpose(2, 1, 0).astype(dtype))

    in_maps = []
    for core in range(NCORES):
        d = core // 4           # 0 = forward, 1 = backward
        g = core % 4
        bsl = slice(g * BC, (g + 1) * BC)
        xc = x[bsl] if d == 0 else x[bsl, ::-1]
        xTc = np.ascontiguousarray(
            xc.transpose(2, 1, 0).reshape(D, T * BC).astype(bf16))
        h0 = np.asarray((h_f if d == 0 else h_b)[bsl], np.float32)
        c0 = np.asarray((c_f if d == 0 else c_b)[bsl], np.float32)
        in_maps.append({
            "xT": xTc,
            "h0T": to_T(h0, bf16),
            "c0T": to_T(c0, np.float32),
            "Wk": Wks[d],
            "Wr": Wrs[d],
            "b": bs[d],
            "selc": selc,
            "sela": sela,
        })
    return in_maps


def kernel(x, h_f, c_f, h_b, c_b, Wk_f, Wr_f, b_f, Wk_b, Wr_b, b_b):
    from concourse.bass_utils import run_bass_kernel_spmd

    if "nc" not in _CACHE:
        _CACHE["nc"] = _build_program()
    nc = _CACHE["nc"]
    in_maps = _make_in_maps(x, h_f, c_f, h_b, c_b, Wk_f, Wr_f, b_f, Wk_b, Wr_b, b_b)

    import os
    trace = os.environ.get("BLSTM_TRACE") == "1"
    tmpdir = os.environ.get("BLSTM_TRACE_DIR") or None
    br = run_bass_kernel_spmd(nc, in_maps, list(range(NCORES)), trace=trace, tmpdir=tmpdir)
    _CACHE["exec_time_ns"] = br.exec_time_ns
    _CACHE["br"] = br
    res = br.results

    out = np.empty((B, T, 2 * U), np.float32)
    for core in range(NCORES):
        d = core // 4
        g = core % 4
        yc = np.asarray(res[core]["yT"], dtype=np.float32)
        yc = yc.transpose(3, 0, 2, 1).reshape(BC, T, UP)[:, :, :U]
        bsl = slice(g * BC, (g + 1) * BC)
        if d == 0:
            out[bsl, :, :U] = yc
        else:
            out[bsl, :, U:] = yc[:, ::-1]
    return out


# revision 29
# speedup vs baseline: 1.0451x; 1.0451x over previous
"""Bidirectional LSTM (B=64, T=256, D=512, U=500) on 8 Trainium2 NeuronCores.

Sharding: 2 directions x 4 batch-groups -> 16 samples per core, one direction
per core. Backward cores receive time-reversed x from the host; the device
program is pure SPMD.

v4 design: recurrence in (128,32) column-tiled PE mode, elementwise tail in
transposed space, and the input GEMM interleaved into the recurrence loop:
  - z bank [128,500]: f@0-15, i@32-47, o@64-79, g@96-111 (bf16 operands),
    built by 4-way concurrent col-tiled matmuls (xz via selector-identity MM
    with start=True, then 4 recurrent u-chunks accumulate)
  - tanh fold: host scales g columns of Wk/Wr/b by 2; ONE packed sigmoid over
    z[0:112] covers all four gates; a constant-1.0 row block (s rows 112-127)
    plus a -1 selector column turns 2*sigmoid into tanh during the transpose
  - transposes: per u-chunk, 4 concurrent piece-matmuls with a [128,64]
    block-selector rhs -> T[128,4,64] = fT|iT|oT|gT
  - T-space tail: gT copied to SBUF on ScalarE; t1 = iT*gT; t2 = fT*cT;
    cT = t1+t2; th = tanh(cT) and hT = oT*th in chunk-halves
  - input GEMM xz = x@Wk + b: 3 m-tiles as a serial prefix, then one
    [128rows x 500cols] bank-tile every other step fills the PE's
    dependency-wait gaps and keeps the HAM clock gate warm; bias-adds run on
    the otherwise-idle GpSimd; xz round-trips through DRAM
"""

import numpy as np

B, T, D, U = 64, 256, 512, 500
G4 = 4 * U            # 2000
NCORES = 8
BC = B // 4           # 16 samples per core
UP = 512              # padded U (4 chunks of 128)
NCH = 4               # u-chunks
CH_W = [128, 128, 128, 116]
CH_LO = [0, 128, 256, 384]
NSL = 500
DCH = 4               # D = 4 chunks of 128 (input GEMM contraction)
MT = (T * BC) // 128  # 32 M-tiles in the input GEMM
NQ = 4 * MT           # 128 GEMM bank-tiles (m-tile x n-bank)
PREFIX_Q = 12         # 3 m-tiles computed before the loop

_CACHE = {}


def _build_program(steps=T):
    import concourse.bass as bass
    import concourse.bacc as bacc
    import concourse.tile as tile
    import concourse.mybir as mybir

    dt = mybir.dt
    AF = mybir.ActivationFunctionType
    f32 = dt.float32
    f32r = dt.float32r
    bf16 = dt.bfloat16

    nc = bacc.Bacc("TRN2")

    xT = nc.dram_tensor("xT", [D, T * BC], bf16, kind="ExternalInput")
    h0T = nc.dram_tensor("h0T", [128, NCH, BC], bf16, kind="ExternalInput")
    c0T = nc.dram_tensor("c0T", [128, NCH, BC], f32, kind="ExternalInput")
    Wk = nc.dram_tensor("Wk", [D, G4], bf16, kind="ExternalInput")  # cols fiog
    Wr = nc.dram_tensor("Wr", [UP, G4], bf16, kind="ExternalInput")  # rows pad
    bv = nc.dram_tensor("b", [G4], f32, kind="ExternalInput")
    selc = nc.dram_tensor("selc", [128, BC], bf16, kind="ExternalInput")
    sela = nc.dram_tensor("sela", [128, 4 * BC], bf16, kind="ExternalInput")
    # yT[t, p, r, b] = h_t[b, 128*r + p]  (junk rows p>=116 for r=3)
    yT = nc.dram_tensor("yT", [T, 128, NCH, BC], bf16, kind="ExternalOutput")
    xz = nc.dram_tensor("xzbuf", [T * BC, G4], bf16)

    with tile.TileContext(nc) as tc:
        with tc.tile_pool(name="persist", bufs=1) as persist, \
             tc.tile_pool(name="gout", bufs=3) as gout, \
             tc.tile_pool(name="state", bufs=2) as st, \
             tc.tile_pool(name="gates", bufs=2) as gt, \
             tc.tile_pool(name="xzin", bufs=4) as xzp, \
             tc.tile_pool(name="gps", bufs=2, space="PSUM") as gps, \
             tc.tile_pool(name="zps", bufs=2, space="PSUM") as zp, \
             tc.tile_pool(name="tps", bufs=2, space="PSUM") as tpp:

            wr_sb = persist.tile([128, NCH, G4], bf16)
            sel_c = persist.tile([128, BC], bf16)
            nc.sync.dma_start(sel_c, selc[:, :])
            sel_a = persist.tile([128, 4 * BC], bf16)
            nc.sync.dma_start(sel_a, sela[:, :])
            wk_sb = persist.tile([128, DCH, G4], bf16)
            for k in range(DCH):
                nc.gpsimd.dma_start(wk_sb[:, k, :], Wk[k * 128:(k + 1) * 128, :])
            XB = (T * BC) // 8
            xtb = []
            for hblk in range(8):
                xtb_t = persist.tile([128, DCH, XB], bf16, name=f"xtb{hblk}")
                xtb.append(xtb_t)
                for k in range(DCH):
                    nc.gpsimd.dma_start(
                        xtb_t[:, k, :],
                        xT[k * 128:(k + 1) * 128, hblk * XB:(hblk + 1) * XB])
            for r in range(NCH):
                nc.gpsimd.dma_start(wr_sb[:, r, :], Wr[r * 128:(r + 1) * 128, :])


            def gemm_bank_tile(q):
                """One [128 x 500] tile of xz = x@Wk (m-tile q//4, bank q%4).

                Column-tiled bf16 (stays in (128,32) mode). The k-order is
                staggered across col groups because start=True clears the
                whole bank's has_written bits: only the very first matmul may
                use it; later groups' k0 land on cleared bits and overwrite.
                Bias is added later by the xz selector matmul (b row in xzt).
                Evacuation is a ScalarE copy issued at the end of the step.
                """
                m, n = q // 4, q % 4
                gb = gps.tile([128, 512], f32, tag="gb", name=f"gb{q}")
                mlo = (m % 4) * 128
                # k staggered across col groups: concurrent tiles then stream
                # DIFFERENT wk addresses (same-address streams halve the rate
                # on the SBUF read ports). start=True clears has_written for
                # [out partitions x whole bank free-range]; groups have
                # disjoint partitions so each primes at its own k=0.
                for s in range(DCH + 3):
                    for j in range(4):
                        k = s - j
                        if k < 0 or k >= DCH:
                            continue
                        nc.tensor.matmul(
                            gb[32 * j:32 * (j + 1), 0:NSL],
                            lhsT=xtb[m // 4][:, k, mlo + 32 * j:mlo + 32 * (j + 1)],
                            rhs=wk_sb[:, k, n * NSL:(n + 1) * NSL],
                            start=(k == 0),
                            stop=(k == DCH - 1),
                            tile_position=(0, 32 * j),
                        )
                so = gout.tile([128, NSL], bf16, tag="so", name=f"so{q}")
                for e in range(4):
                    el, eh = 125 * e, 125 * (e + 1)
                    nc.vector.tensor_copy(so[:, el:eh], gb[:, el:eh])
                nc.sync.dma_start(
                    xz[m * 128:(m + 1) * 128, n * NSL:(n + 1) * NSL], so)

            # ---- init: memset junk regions of rotating buffers once ----
            for _ in range(2):
                zi = zp.tile([128, 256], f32, tag="zA")
                nc.vector.memset(zi, 0.0)
                zi2 = zp.tile([128, NSL - 256], f32, tag="zB")
                nc.vector.memset(zi2, 0.0)
            for _ in range(2):
                ti = tpp.tile([128, NCH, 4 * BC], f32, tag="T")
                nc.vector.memset(ti, 0.0)
            for _ in range(2):
                si = gt.tile([128, NSL], bf16, tag="s")
                nc.vector.memset(si, 0.0)
                nc.vector.memset(si[96:128, :], 1.0)
            for _ in range(4):
                xi = xzp.tile([128, G4], bf16, tag="xz")
                nc.vector.memset(xi, 0.0)
                # bias lives in row 16 of every xz staging buffer; the
                # selector's ones-row adds it to all batch rows
                nc.gpsimd.dma_start(xi[16:17, :], bv[:].unsqueeze(0))

            cT_prev = st.tile([128, NCH, BC], f32, tag="cT")
            nc.sync.dma_start(cT_prev, c0T[:, :, :])
            hT_prev = st.tile([128, NCH, BC], bf16, tag="hT")
            nc.sync.dma_start(hT_prev, h0T[:, :, :])

            # ---- GEMM prefix ----
            for q in range(PREFIX_Q):
                gemm_bank_tile(q)
            next_q = PREFIX_Q

            xzt = xzp.tile([128, G4], bf16, tag="xz")
            nc.sync.dma_start(xzt[0:BC, :], xz[0:BC, :])

            # z built in column halves A=[0:256], B=[256:500]: sigmoid's first
            # slice starts right after the A half-burst
            HALVES = [(0, 256), (256, NSL)]

            def xz_slot(zh, xzt_, c0, c1):
                for g in range(4):
                    nc.tensor.matmul(
                        zh[32 * g:32 * g + BC, 0:c1 - c0],
                        lhsT=sel_c,
                        rhs=xzt_[:, g * NSL + c0:g * NSL + c1],
                        start=True, stop=False,
                        tile_position=(0, 32 * g),
                    )

            def rec_slot(zh, hT_, r, c0, c1):
                for g in range(4):
                    nc.tensor.matmul(
                        zh[32 * g:32 * g + BC, 0:c1 - c0],
                        lhsT=hT_[:, r, :],
                        rhs=wr_sb[:, r, g * NSL + c0:g * NSL + c1],
                        start=False, stop=(r == NCH - 1),
                        tile_position=(0, 32 * g),
                    )

            def new_z():
                return (zp.tile([128, 256], f32, tag="zA", name="zA"),
                        zp.tile([128, NSL - 256], f32, tag="zB", name="zB"))

            z = new_z()
            for (c0, c1), zh in zip(HALVES, z):
                xz_slot(zh, xzt, c0, c1)
                for r in range(NCH):
                    rec_slot(zh, hT_prev, r, c0, c1)

            for t in range(steps):
                if t + 1 < steps:
                    xzt_n = xzp.tile([128, G4], bf16, tag="xz")
                    nc.sync.dma_start(
                        xzt_n[0:BC, :], xz[(t + 1) * BC:(t + 2) * BC, :])

                # packed sigmoid over all 4 gates, one op per z half-tile
                s_sb = gt.tile([128, NSL], bf16, tag="s")
                nc.scalar.activation(
                    s_sb[0:112, 0:256], z[0][0:112, :], AF.Sigmoid)
                nc.scalar.activation(
                    s_sb[0:112, 256:NSL], z[1][0:112, :], AF.Sigmoid)

                if t + 1 < steps:
                    z_n = new_z()
                    for (c0, c1), zh in zip(HALVES, z_n):
                        xz_slot(zh, xzt_n, c0, c1)

                # transposes: per u-chunk, 4 concurrent piece-MMs
                Tt = tpp.tile([128, NCH, 4 * BC], f32, tag="T")
                for r in range(NCH):
                    lo, w = CH_LO[r], CH_W[r]
                    for j in range(4):
                        pw = min(32, w - 32 * j)
                        if pw <= 0:
                            continue
                        pl = lo + 32 * j
                        nc.tensor.matmul(
                            Tt[32 * j:32 * j + pw, r, :],
                            lhsT=s_sb[:, pl:pl + pw],
                            rhs=sel_a,
                            start=True, stop=True,
                            tile_position=(0, 32 * j),
                        )

                # T-space tail, sliced into chunk-halves so the first half's
                # hT unblocks next step's rec chunks 0,1 early
                fT = Tt[:, :, 0:BC]
                iT = Tt[:, :, BC:2 * BC]
                oT = Tt[:, :, 2 * BC:3 * BC]
                gT = Tt[:, :, 3 * BC:4 * BC]
                g_sb = gt.tile([128, NCH, BC], f32, tag="gsb")
                nc.vector.tensor_copy(g_sb, gT)
                t2 = gt.tile([128, NCH, BC], f32, tag="t2")
                nc.vector.tensor_mul(t2, fT, cT_prev)
                t1 = gt.tile([128, NCH, BC], f32, tag="t1")
                nc.vector.tensor_mul(t1, iT, g_sb)
                cT_new = st.tile([128, NCH, BC], f32, tag="cT")
                nc.vector.tensor_add(cT_new[:, 0:2, :], t1[:, 0:2, :],
                                     t2[:, 0:2, :])
                nc.vector.tensor_add(cT_new[:, 2:4, :], t1[:, 2:4, :],
                                     t2[:, 2:4, :])
                th = gt.tile([128, NCH, BC], f32, tag="th")
                hT_new = st.tile([128, NCH, BC], bf16, tag="hT")
                nc.scalar.activation(th[:, 0:2, :], cT_new[:, 0:2, :], AF.Tanh)
                nc.vector.tensor_mul(hT_new[:, 0:2, :], oT[:, 0:2, :],
                                     th[:, 0:2, :])
                nc.scalar.activation(th[:, 2:4, :], cT_new[:, 2:4, :], AF.Tanh)
                nc.vector.tensor_mul(hT_new[:, 2:4, :], oT[:, 2:4, :],
                                     th[:, 2:4, :])

                nc.sync.dma_start(yT[t], hT_new)

                if t + 1 < steps:
                    for (c0, c1), zh in zip(HALVES, z_n):
                        for r in range(NCH):
                            rec_slot(zh, hT_new, r, c0, c1)
                    z = z_n
                    xzt = xzt_n

                # one GEMM bank-tile every other step; issued last so its PE
                # matmuls and DVE bias-add drain into idle time
                if t % 2 == 0 and next_q < NQ:
                    gemm_bank_tile(next_q)
                    next_q += 1

                cT_prev = cT_new
                hT_prev = hT_new
    nc.finalize()
    return nc


# Keras gate order [i, f, g, o] -> kernel order [f, i, o, g]
_PERM = np.concatenate([
    np.arange(U, 2 * U),      # f
    np.arange(0, U),          # i
    np.arange(3 * U, 4 * U),  # o
    np.arange(2 * U, 3 * U),  # g
])
# scale applied per gate column block after permutation: g block doubled
_GSCALE = np.concatenate([
    np.ones(3 * U, np.float32), np.full(U, 2.0, np.float32)])


def _make_in_maps(x, h_f, c_f, h_b, c_b, Wk_f, Wr_f, b_f, Wk_b, Wr_b, b_b):
    import ml_dtypes
    bf16 = ml_dtypes.bfloat16

    x = np.ascontiguousarray(np.asarray(x, np.float32))
    Wks = [np.ascontiguousarray(
               (np.asarray(Wk_f, np.float32)[:, _PERM] * _GSCALE).astype(bf16)),
           np.ascontiguousarray(
               (np.asarray(Wk_b, np.float32)[:, _PERM] * _GSCALE).astype(bf16))]
    Wrs = []
    for Wr in (Wr_f, Wr_b):
        w = np.asarray(Wr, np.float32)[:, _PERM] * _GSCALE
        wp = np.zeros((UP, G4), np.float32)
        wp[:U] = w
        Wrs.append(np.ascontiguousarray(wp.astype(bf16)))
    bs = [np.ascontiguousarray(np.asarray(b_f, np.float32)[_PERM] * _GSCALE),
          np.ascontiguousarray(np.asarray(b_b, np.float32)[_PERM] * _GSCALE)]

    selc = np.zeros((128, BC), np.float32)
    for n in range(BC):
        selc[n, n] = 1.0
    selc[16, :] = 1.0  # adds the b row (xzt row 16) to every batch row
    sela = np.zeros((128, 4 * BC), np.float32)
    for n in range(BC):
        sela[n, n] = 1.0            # fT
        sela[32 + n, BC + n] = 1.0  # iT
        sela[64 + n, 2 * BC + n] = 1.0  # oT
        sela[96 + n, 3 * BC + n] = 2.0  # 2*sg ...
        sela[112, 3 * BC + n] = -1.0  # ... - 1 (s_sb rows 112+ are const 1.0)
    selc = selc.astype(bf16)
    sela = sela.astype(bf16)

    def to_T(hv, dtype):
        hp = np.zeros((BC, UP), np.float32)
        hp[:, :U] = hv
        return np.ascontiguousarray(
            hp.reshape(BC, NCH, 128).transpose(2, 1, 0).astype(dtype))

    in_maps = []
    for core in range(NCORES):
        d = core // 4           # 0 = forward, 1 = backward
        g = core % 4
        bsl = slice(g * BC, (g + 1) * BC)
        xc = x[bsl] if d == 0 else x[bsl, ::-1]
        xTc = np.ascontiguousarray(
            xc.transpose(2, 1, 0).reshape(D, T * BC).astype(bf16))
        h0 = np.asarray((h_f if d == 0 else h_b)[bsl], np.float32)
        c0 = np.asarray((c_f if d == 0 else c_b)[bsl], np.float32)
        in_maps.append({
            "xT": xTc,
            "h0T": to_T(h0, bf16),
            "c0T": to_T(c0, np.float32),
            "Wk": Wks[d],
            "Wr": Wrs[d],
            "b": bs[d],
            "selc": selc,
            "sela": sela,
        })
    return in_maps


def kernel(x, h_f, c_f, h_b, c_b, Wk_f, Wr_f, b_f, Wk_b, Wr_b, b_b):
    from concourse.bass_utils import run_bass_kernel_spmd

    if "nc" not in _CACHE:
        _CACHE["nc"] = _build_program()
    nc = _CACHE["nc"]
    in_maps = _make_in_maps(x, h_f, c_f, h_b, c_b, Wk_f, Wr_f, b_f, Wk_b, Wr_b, b_b)

    import os
    trace = os.environ.get("BLSTM_TRACE") == "1"
    tmpdir = os.environ.get("BLSTM_TRACE_DIR") or None
    br = run_bass_kernel_spmd(nc, in_maps, list(range(NCORES)), trace=trace, tmpdir=tmpdir)
    _CACHE["exec_time_ns"] = br.exec_time_ns
    _CACHE["br"] = br
    res = br.results

    out = np.empty((B, T, 2 * U), np.float32)
    for core in range(NCORES):
        d = core // 4
        g = core % 4
        yc = np.asarray(res[core]["yT"], dtype=np.float32)
        yc = yc.transpose(3, 0, 2, 1).reshape(BC, T, UP)[:, :, :U]
        bsl = slice(g * BC, (g + 1) * BC)
        if d == 0:
            out[bsl, :, :U] = yc
        else:
            out[bsl, :, U:] = yc[:, ::-1]
    return out
